# revision 1
# baseline (speedup 1.0000x reference)
"""Causal single-head attention on 8 Trainium2 NeuronCores.

Problem (hardcoded): x [8, 2048, 2048] f32; Wq/Wk/Wv [2048, 128]; bq/bk/bv [128].
out[b] = softmax_causal((x[b]Wq + bq)(x[b]Wk + bk)^T / sqrt(128)) (x[b]Wv + bv)

Sharding: data-parallel over batch — core b computes batch element b entirely
on-chip. Weights replicated. No collectives.

Per-core pipeline. QKV path runs in fp16 (x and W converted on the host;
PE transposes stream at 1 cyc/row vs 2 for fp32, matmuls accumulate in fp32
PSUM); softmax denominators, reciprocal, normalization, and the final output
stay fp32/f32r. Measured rel error 5.4e-4 vs the fp32 reference.
  Phase A (per 512-wide T-chunk):
    - DMA x rows (fp16); PE-transpose 128x128 blocks -> x^T (fp16), rotating
      4 PSUM banks so bank-turnaround hides under the streams
    - QT/KT/VT [H, T] += W-tile^T @ x^T (fp16 in, fp32 PSUM accum), rotating
      the 3 projection accumulator banks (e outer / head inner)
    - fp32 bias added during the DVE PSUM->SBUF copy (output fp16)
    - V^T re-transposed to natural V [T, H] (PV stationary operand)
  Phase B (per 512-wide q-block, k-tiles causally restricted):
    - S^T tile = KT-tile-stationary @ QT-moving -> fp32 PSUM
    - diagonal tiles: += identity_bf16 @ trimask_bf16 (-1e30 below diag of
      [k, q]); software-pipelined so the mask add lands 2 PE ops after its
      scores matmul (bank alternation) and exp runs 2 iterations behind
    - P^T = exp(S^T / sqrt(128)) via ACT, PSUM -> fp16 SBUF
    - denom row += ones16^T @ P^T ; out^T += V-tile^T @ P^T (fp32 PSUM)
    - 1/denom: DVE copy -> f32r, DVE reciprocal, then ONE K=1 matmul
      broadcasts the reciprocal row across all 128 H partitions (epilogue
      copies kept off ACT, whose exp gates the PE's dn/PV matmuls)
    - one DVE tensor_mul normalizes out^T; DMA to DRAM as [H, T]
      (kernel() un-transposes on the host for free).
"""

import sys

sys.path.insert(0, "/opt/trn_rl_repo")

from contextlib import ExitStack

import numpy as np

import concourse.mybir as mybir
import concourse.tile as tile
from concourse import bacc
from concourse.bass_utils import run_bass_kernel_spmd

F32 = mybir.dt.float32
F32R = mybir.dt.float32r
F16 = mybir.dt.float16
BF16 = mybir.dt.bfloat16
AF = mybir.ActivationFunctionType

B, T, E, H = 8, 2048, 2048, 128
NT = T // 128  # 16 t-tiles
NE = E // 128  # 16 e-tiles
CH = 512  # T-chunk / q-block width
NCH = T // CH  # 4
TPC = CH // 128  # 4 t-tiles per chunk
SCALE = 1.0 / float(np.sqrt(H))
NEG = -1.0e30


def build_nc(loop_n=1):
    nc = bacc.Bacc("TRN2", target_bir_lowering=False, debug=False)

    x_d = nc.dram_tensor("x", [T, E], F16, kind="ExternalInput").ap()
    w_d = {
        n: nc.dram_tensor(f"w{n}", [E, H], F16, kind="ExternalInput").ap()
        for n in "qkv"
    }
    b_d = {
        n: nc.dram_tensor(f"b{n}", [H, 1], F32, kind="ExternalInput").ap()
        for n in "qkv"
    }
    ident_d = nc.dram_tensor("ident", [128, 128], F32, kind="ExternalInput").ap()
    ident16_d = nc.dram_tensor("ident16", [128, 128], F16, kind="ExternalInput").ap()
    ones16_d = nc.dram_tensor("ones16", [128, 1], F16, kind="ExternalInput").ap()
    identb_d = nc.dram_tensor("identb", [128, 128], BF16, kind="ExternalInput").ap()
    maskb_d = nc.dram_tensor("maskb", [128, 128], BF16, kind="ExternalInput").ap()
    ones_d = nc.dram_tensor("ones", [128, 2], F32, kind="ExternalInput").ap()
    onesr_d = nc.dram_tensor("onesr", [1, 128], F32, kind="ExternalInput").ap()
    # output stored transposed [H, T]; host un-transposes (free on HW)
    out_d = nc.dram_tensor("out", [H, T], F32, kind="ExternalOutput").ap()

    x_t = x_d.rearrange("(n p) e -> n p e", p=128)

    with tile.TileContext(nc) as tc, ExitStack() as ctx:
        if loop_n > 1:
            ctx.enter_context(tc.For_i(0, loop_n, 1))
        const = ctx.enter_context(tc.tile_pool(name="const", bufs=1))
        wpool = ctx.enter_context(tc.tile_pool(name="w", bufs=1))
        qkvt = ctx.enter_context(tc.tile_pool(name="qkvt", bufs=1))

        ident = const.tile([128, 128], F32, tag="ident")
        ident16 = const.tile([128, 128], F16, tag="ident16")
        ones16 = const.tile([128, 1], F16, tag="ones16")
        identb = const.tile([128, 128], BF16, tag="identb")
        maskb = const.tile([128, 128], BF16, tag="maskb")
        ones = const.tile([128, 2], F32R, tag="ones")
        onesr = const.tile([1, 128], F32R, tag="onesr")
        nc.sync.dma_start(ident, ident_d)
        nc.sync.dma_start(ident16, ident16_d)
        nc.sync.dma_start(ones16, ones16_d)
        nc.sync.dma_start(identb, identb_d)
        nc.sync.dma_start(maskb, maskb_d)
        nc.sync.dma_start(ones, ones_d.bitcast(F32R))
        nc.sync.dma_start(onesr, onesr_d.bitcast(F32R))
        bias = {}
        for n in "qkv":
            bias[n] = const.tile([128, 1], F32, tag=f"b{n}", name=f"bias_{n}")
            nc.sync.dma_start(bias[n], b_d[n])
        # chunk-0 x tiles first: the first PE transposes wait on these, and
        # queuing them behind 3 MB of weight loads costs ~9us of idle PE
        xn0 = []
        first_pool = tc.tile_pool(name="xnat", bufs=8)
        w_sb = {}
        for n in "qkv":
            w_sb[n] = wpool.tile([128, NE * 128], F16, tag=f"w{n}", name=f"w_{n}")

        # persistent transposed projections [H, T] and natural V [T, H]
        QT = qkvt.tile([128, T], F16, tag="QT")
        KT = qkvt.tile([128, T], F16, tag="KT")
        VT = qkvt.tile([128, T], F16, tag="VT")
        Vn = qkvt.tile([128, T], F16, tag="Vn")  # slice i = V[128i:128(i+1), :]
        dest = {"q": QT, "k": KT, "v": VT}

        # ---------------- Phase A: transpose x + projections ----------------
        with ExitStack() as actx:
            xnat = actx.enter_context(first_pool)
            for t in range(TPC):
                xt_tile = xnat.tile([128, E], F16, tag="xn", name=f"xn0_{t}")
                nc.sync.dma_start(xt_tile, x_t[t])
                xn0.append(xt_tile)
            for n in "qkv":
                nc.sync.dma_start(
                    w_sb[n].rearrange("p (n m) -> p n m", m=128),
                    w_d[n].rearrange("(n p) m -> p n m", p=128),
                )
            xtp = actx.enter_context(tc.tile_pool(name="xt", bufs=20))
            psx = actx.enter_context(tc.tile_pool(name="psx", bufs=1, space="PSUM"))
            psp = actx.enter_context(tc.tile_pool(name="psp", bufs=1, space="PSUM"))

            for c in range(NCH):
                if c == 0:
                    xn = xn0
                else:
                    xn = []
                    for t in range(TPC):
                        xt_tile = xnat.tile([128, E], F16, tag="xn")
                        nc.sync.dma_start(xt_tile, x_t[TPC * c + t])
                        xn.append(xt_tile)
                xts = []
                # four e-tiles at a time; consecutive transposes rotate four
                # PSUM banks so bank-turnaround fully hides (same-bank gap =
                # 3 streams > ~172ns turnaround)
                for e0 in range(0, NE, 4):
                    tps = [
                        psx.tile([128, CH], F16, tag=f"tp{k}", name=f"tp{k}")
                        for k in range(4)
                    ]
                    for t in range(TPC):
                        for k in range(4):
                            nc.tensor.transpose(
                                tps[k][:, 128 * t : 128 * (t + 1)],
                                xn[t][:, 128 * (e0 + k) : 128 * (e0 + k + 1)],
                                ident16,
                            )
                    for k in range(4):
                        xk = xtp.tile([128, CH], F16, tag="xt", name=f"x{k}")
                        if k % 2 == 0:
                            nc.scalar.activation(xk, tps[k], AF.Copy)
                        else:
                            nc.vector.tensor_copy(xk, tps[k])
                        xts.append(xk)

                # e outer / proj inner: consecutive matmuls cycle 3 PSUM banks
                pp = {}
                for n in "qkv":
                    pp[n] = psp.tile([128, CH], F32, tag=f"pp{n}", name=f"pp{n}")
                for e in range(NE):
                    for n in "qkv":
                        nc.tensor.matmul(
                            pp[n],
                            w_sb[n][:, 128 * e : 128 * (e + 1)],
                            xts[e],
                            start=(e == 0),
                            stop=(e == NE - 1),
                        )
                for n in "qkv":
                    nc.vector.tensor_scalar_add(
                        dest[n][:, CH * c : CH * (c + 1)], pp[n], bias[n]
                    )

                # natural-layout V for the PV stationary operand; vpa holds
                # t-tiles {0,1}, vpb {2,3}; emission alternates PSUM banks
                vpa = psx.tile([128, 256], F16, tag="tp0", name="vpa")
                vpb = psx.tile([128, 256], F16, tag="tp1", name="vpb")
                for m in range(2):
                    nc.tensor.transpose(
                        vpa[:, 128 * m : 128 * (m + 1)],
                        VT[:, CH * c + 128 * m : CH * c + 128 * (m + 1)],
                        ident16,
                    )
                    nc.tensor.transpose(
                        vpb[:, 128 * m : 128 * (m + 1)],
                        VT[:, CH * c + 128 * (m + 2) : CH * c + 128 * (m + 3)],
                        ident16,
                    )
                nc.scalar.activation(Vn[:, CH * c : CH * c + 256], vpa, AF.Copy)
                nc.vector.tensor_copy(Vn[:, CH * c + 256 : CH * (c + 1)], vpb)

        # ---------------- Phase B: causal attention ----------------
        with ExitStack() as bctx:
            pss = bctx.enter_context(tc.tile_pool(name="pss", bufs=3, space="PSUM"))
            pso = bctx.enter_context(tc.tile_pool(name="pso", bufs=2, space="PSUM"))
            psd = bctx.enter_context(tc.tile_pool(name="psd", bufs=1, space="PSUM"))
            pst = bctx.enter_context(tc.tile_pool(name="pst", bufs=2, space="PSUM"))
            ppool = bctx.enter_context(tc.tile_pool(name="pp", bufs=3))
            opool = bctx.enter_context(tc.tile_pool(name="op", bufs=2))
            dpool = bctx.enter_context(tc.tile_pool(name="dp", bufs=2))
            rpool = bctx.enter_context(tc.tile_pool(name="rp", bufs=4))
            fpool = bctx.enter_context(tc.tile_pool(name="fp", bufs=4))

            for j in range(NCH):
                ni = 4 * j + 4  # number of causal k-tiles for this q-block
                outp = pso.tile([128, CH], F32, tag="outp")
                dn = psd.tile([1, CH], F32, tag="dn")

                # software-pipelined: scores run ahead; mask lands after the
                # previous iteration's dn/PV so consecutive PE ops alternate
                # PSUM banks
                stage = []  # (i, c0, p)
                masks = []  # deferred mask adds: (sps, c0)
                exps = []   # deferred exp emits: (i, c0, sps, diag)

                def emit_s(i):
                    c0 = max(0, 128 * (i - 4 * j))
                    sps = pss.tile([128, CH], F32, tag="sps", name="sps")
                    diag = i >= 4 * j
                    nc.tensor.matmul(
                        sps[:, c0:],
                        KT[:, 128 * i : 128 * (i + 1)],
                        QT[:, CH * j + c0 : CH * (j + 1)],
                        start=True,
                        stop=not diag,
                    )
                    exps.append((i, c0, sps, diag))

                def emit_mask_exp():
                    if not exps:
                        return
                    i, c0, sps, diag = exps.pop(0)
                    if diag:
                        nc.tensor.matmul(
                            sps[:, c0 : c0 + 128],
                            identb,
                            maskb,
                            start=False,
                            stop=True,
                            skip_group_check=True,
                        )
                    p = ppool.tile([128, CH], F16, tag="p", name="p")
                    nc.scalar.activation(p[:, c0:], sps[:, c0:], AF.Exp, scale=SCALE)
                    stage.append((i, c0, p))

                def emit_accum(i, c0, p):
                    nc.tensor.matmul(
                        dn[0:1, c0:],
                        ones16,
                        p[:, c0:],
                        start=(i == 0),
                        stop=(i == ni - 1),
                        skip_group_check=True,
                    )
                    nc.tensor.matmul(
                        outp[:, c0:],
                        Vn[:, 128 * i : 128 * (i + 1)],
                        p[:, c0:],
                        start=(i == 0),
                        stop=(i == ni - 1),
                        skip_group_check=True,
                    )

                for i in range(ni):
                    emit_s(i)
                    if len(stage) >= 2:
                        emit_accum(*stage.pop(0))
                    emit_mask_exp()
                while stage or exps:
                    if stage:
                        emit_accum(*stage.pop(0))
                    emit_mask_exp()

                # epilogue copies on DVE: ACT's exp is co-critical with the
                # PE in phase B (dn/PV wait on it), DVE has slack
                dn_sb = dpool.tile([1, CH], F32R, tag="dn_sb")
                nc.vector.tensor_copy(dn_sb, dn)
                ot_sb = opool.tile([128, CH], F32, tag="ot_sb")
                nc.vector.tensor_copy(ot_sb, outp)

                # 1/denom broadcast across all H partitions via K=1 matmul,
                # then one fused normalize; output stays [H, q] (host
                # un-transposes the DRAM tensor)
                recip = rpool.tile([1, CH], F32R, tag="recip")
                with nc.allow_low_precision(reason="f32r is 4-byte; feeds matmul"):
                    nc.vector.reciprocal(recip, dn_sb.bitcast(F32))
                rb = pst.tile([128, CH], F32, tag="pt")
                nc.tensor.matmul(rb, onesr, recip, start=True, stop=True)
                o_sb = fpool.tile([128, CH], F32, tag="o_sb")
                nc.vector.tensor_mul(o_sb, ot_sb, rb)
                nc.sync.dma_start(out_d[:, CH * j : CH * (j + 1)], o_sb)

    nc.compile()
    return nc


_CACHE = {}


def make_shared(inputs):
    """Per-core in_map entries shared across cores: weights, biases, consts."""
    import ml_dtypes

    shared = {
        "ident": np.eye(128, dtype=np.float32),
        "identb": np.eye(128, dtype=ml_dtypes.bfloat16),
        # maskb[k, q] = 0 if k <= q else NEG   (S^T layout: rows=k, cols=q)
        "maskb": np.tril(np.full((128, 128), NEG, np.float32), -1).astype(
            ml_dtypes.bfloat16
        ),
        "ones": np.ones((128, 2), np.float32),
        "onesr": np.ones((1, 128), np.float32),
        "ident16": np.eye(128, dtype=np.float16),
        "ones16": np.ones((128, 1), np.float16),
    }
    for n in "qkv":
        shared[f"w{n}"] = np.ascontiguousarray(inputs[f"W{n}"], dtype=np.float32).astype(
            np.float16
        )
        shared[f"b{n}"] = np.ascontiguousarray(
            inputs[f"b{n}"], dtype=np.float32
        ).reshape(H, 1)
    return shared


def kernel(**inputs):
    x = np.ascontiguousarray(inputs["x"], dtype=np.float32)
    assert x.shape == (B, T, E)

    if "nc" not in _CACHE:
        _CACHE["nc"] = build_nc()
    nc = _CACHE["nc"]

    shared = make_shared(inputs)
    x16 = x.astype(np.float16)
    in_maps = [dict(shared, x=np.ascontiguousarray(x16[b])) for b in range(B)]
    res = run_bass_kernel_spmd(nc, in_maps, core_ids=list(range(B)))
    return np.stack(
        [np.ascontiguousarray(r["out"].T) for r in res.results], axis=0
    )


if __name__ == "__main__":
    rng = np.random.default_rng(0)
    ins = {
        "x": rng.standard_normal((B, T, E)).astype(np.float32),
        **{f"W{n}": rng.standard_normal((E, H)).astype(np.float32) / 45 for n in "qkv"},
        **{f"b{n}": rng.standard_normal((H,)).astype(np.float32) / 45 for n in "qkv"},
    }
    out = kernel(**ins)
    print(out.shape, out.dtype)

